# revision 1
# baseline (speedup 1.0000x reference)
"""Trainium2 Bass kernel: cross-modal channel attention.

Math (per batch b), with G the static [L, S] linear-interp matrix:
    q    = img_feat[b] reshaped [C, S]            (C=768, S=1024, L=77, D=512)
    tp   = text_feat[b] @ W_txt                   [L, C]
    t    = tp^T @ G                               [C, S]   (never materialized)
    logits^T = t @ q^T = tp^T @ (G @ q^T)         [Cj, Ci]  -- factored via L
    E^T  = exp(logits^T * S^-0.5)                 [Cj, Ci]
    Z_i  = sum_j E^T[j, i]   (ones-column matmuls)
    outA = E @ t = (tp @ E^T)^T @ G               [Ci, S]   -- factored via L
    out  = q + (gamma / Z_i) * outA               [C, S]

Sharding: data-parallel over batch across 8 cores (4 batches/core);
W_txt, G, gamma replicated.  Matmuls run in float32r (TF32 PE fast path,
1 cycle/row); fp32r operands are produced by rounding PSUM->SBUF copies /
activations.  The residual q stays exact fp32.
"""

import sys

sys.path.insert(0, "/opt/trn_rl_repo")

from contextlib import ExitStack

import numpy as np

import concourse.bacc as bacc
import concourse.mybir as mybir
import concourse.tile as tile
from concourse.bass_utils import run_bass_kernel_spmd
from concourse.masks import make_identity

B, C, HH, WW = 32, 768, 32, 32
S = HH * WW
L, D = 77, 512
N_CORES = 8
B_CORE = B // N_CORES
P = 128
CT, ST, DT = C // P, S // P, D // P
F32 = mybir.dt.float32
F32R = mybir.dt.float32r
SCALE = float(S) ** -0.5
EXP = mybir.ActivationFunctionType.Exp
MULT = mybir.AluOpType.mult
ADD = mybir.AluOpType.add


def _round_tf32(x):
    """Round fp32 -> tf32-representable (10-bit mantissa, round-to-nearest-even)."""
    u = np.ascontiguousarray(x, dtype=np.float32).view(np.uint32)
    r = (u + np.uint32(0x0FFF) + ((u >> np.uint32(13)) & np.uint32(1))) & np.uint32(
        0xFFFFE000
    )
    return r.view(np.float32)


def _interp_matrix():
    """G[l, s] such that (tp^T @ G)[c, s] == linear_interp(tp^T, S)[c, s]."""
    src = np.clip(
        (np.arange(S, dtype=np.float32) + np.float32(0.5)) * np.float32(L / S)
        - np.float32(0.5),
        np.float32(0.0),
        np.float32(L - 1),
    )
    i0 = np.floor(src).astype(np.int32)
    i1 = np.minimum(i0 + 1, L - 1)
    w = (src - i0.astype(np.float32)).astype(np.float32)
    g = np.zeros((L, S), dtype=np.float32)
    g[i0, np.arange(S)] += np.float32(1.0) - w
    g[i1, np.arange(S)] += w
    return g


def _build():
    nc = bacc.Bacc("TRN2", target_bir_lowering=False, debug=False)
    img = nc.dram_tensor("img", [B_CORE, C, S], F32, kind="ExternalInput").ap()
    txt = nc.dram_tensor("txt", [B_CORE, L, D], F32, kind="ExternalInput").ap()
    wt = nc.dram_tensor("wt", [D, C], F32R, kind="ExternalInput").ap()
    g = nc.dram_tensor("g", [L, S], F32R, kind="ExternalInput").ap()
    gt = nc.dram_tensor("gt", [S, L], F32R, kind="ExternalInput").ap()
    gamma = nc.dram_tensor("gamma128", [P, 1], F32, kind="ExternalInput").ap()
    out = nc.dram_tensor("out", [B_CORE, C, S], F32, kind="ExternalOutput").ap()

    with ExitStack() as ctx:
        tc = ctx.enter_context(tile.TileContext(nc))
        consts = ctx.enter_context(tc.tile_pool(name="consts", bufs=1))
        q_pool = ctx.enter_context(tc.tile_pool(name="q", bufs=2))
        txt_pool = ctx.enter_context(tc.tile_pool(name="txtp", bufs=2))
        small = ctx.enter_context(tc.tile_pool(name="small", bufs=2))
        qtb_pool = ctx.enter_context(tc.tile_pool(name="qtb", bufs=3))
        et_pool = ctx.enter_context(tc.tile_pool(name="et", bufs=2))
        outp = ctx.enter_context(tc.tile_pool(name="outp", bufs=2))
        zp = ctx.enter_context(tc.tile_pool(name="zp", bufs=3))
        # PSUM: small pool 2x1 bank + big pool 2x3 banks = 8 banks total.
        ps_small = ctx.enter_context(tc.tile_pool(name="ps_s", bufs=2, space="PSUM"))
        ps_big = ctx.enter_context(tc.tile_pool(name="ps_b", bufs=2, space="PSUM"))

        w_sb = consts.tile([P, DT, C], F32R)
        nc.sync.dma_start(w_sb[:], wt.rearrange("(k p) c -> p k c", p=P))
        g_sb = consts.tile([P, S], F32R)
        nc.sync.dma_start(g_sb[0:L, :], g)
        gt_sb = consts.tile([P, ST, L], F32R)
        nc.sync.dma_start(gt_sb[:], gt.rearrange("(st p) l -> p st l", p=P))
        gamma_sb = consts.tile([P, 1], F32)
        nc.sync.dma_start(gamma_sb[:], gamma)
        ident = consts.tile([P, P], F32)
        make_identity(nc, ident[:])
        # f32r memset/affine_select fail codegen -> produce via rounding copies
        ident_r = consts.tile([P, P], F32R)
        nc.vector.tensor_copy(ident_r[:], ident[:])
        ones_f = consts.tile([P, 2], F32)
        nc.gpsimd.memset(ones_f[:], 1.0)
        ones_sb = consts.tile([P, 2], F32R)
        nc.vector.tensor_copy(ones_sb[:], ones_f[:])

        for b in range(B_CORE):
            q_sb = q_pool.tile([P, CT, S], F32, tag="q")
            nc.sync.dma_start(q_sb[:], img[b].rearrange("(ct p) s -> p ct s", p=P))
            txt_sb = txt_pool.tile([P, D], F32, tag="txt")
            nc.sync.dma_start(txt_sb[0:L, :], txt[b])

            # text^T [D, L] via fp32 PE transposes; rounding copy -> f32r
            ps_tt = ps_small.tile([P, DT, P], F32, tag="ps")
            for k in range(DT):
                nc.tensor.transpose(
                    ps_tt[:, k, 0:L],
                    txt_sb[0:L, k * P : (k + 1) * P],
                    ident[0:L, 0:L],
                )
            txtT_sb = small.tile([P, DT, P], F32R, tag="txtT")
            nc.vector.tensor_copy(txtT_sb[:, :, 0:L], ps_tt[:, :, 0:L])

            # tp = text @ W_txt  [L, C]
            tp_sb = small.tile([P, C], F32R, tag="tp")
            ps_a = ps_small.tile([P, 512], F32, tag="ps")
            for k in range(DT):
                nc.tensor.matmul(
                    ps_a[0:L, :],
                    txtT_sb[:, k, 0:L],
                    w_sb[:, k, 0:512],
                    start=(k == 0),
                    stop=(k == DT - 1),
                )
            nc.scalar.copy(tp_sb[0:L, 0:512], ps_a[0:L, :])
            ps_b2 = ps_small.tile([P, 512], F32, tag="ps")
            for k in range(DT):
                nc.tensor.matmul(
                    ps_b2[0:L, 0:256],
                    txtT_sb[:, k, 0:L],
                    w_sb[:, k, 512:768],
                    start=(k == 0),
                    stop=(k == DT - 1),
                )
            nc.scalar.copy(tp_sb[0:L, 512:768], ps_b2[0:L, 0:256])

            # tp^T [C, L] via f32r PE transposes of tp
            # tp^T via regular matmul against identity (fp32r dst must be even -> N=78)
            ps_tp = ps_small.tile([P, CT, 80], F32, tag="ps")
            for jt in range(CT):
                nc.tensor.matmul(
                    ps_tp[:, jt, 0 : L + 1],
                    tp_sb[0:L, jt * P : (jt + 1) * P],
                    ident_r[0:L, 0 : L + 1],
                    start=True,
                    stop=True,
                )
            tpT_sb = small.tile([P, CT, 80], F32R, tag="tpT")
            nc.vector.tensor_copy(tpT_sb[:, :, 0:L], ps_tp[:, :, 0:L])

            # q^T blocks (streamed) + GQT = G @ q^T  [L, C] accumulated over S
            ps_gqt = ps_big.tile([P, 1025], F32, tag="psb")
            for st in range(ST):
                ps1 = ps_small.tile([P, 512], F32, tag="ps")
                for ct in range(4):
                    nc.tensor.transpose(
                        ps1[:, ct * P : (ct + 1) * P],
                        q_sb[:, ct, st * P : (st + 1) * P],
                        ident[:],
                    )
                ps2 = ps_small.tile([P, 512], F32, tag="ps")
                for ct in range(4, 6):
                    nc.tensor.transpose(
                        ps2[:, (ct - 4) * P : (ct - 3) * P],
                        q_sb[:, ct, st * P : (st + 1) * P],
                        ident[:],
                    )
                qtb = qtb_pool.tile([P, C], F32R, tag="qtb")
                if st % 2 == 0:
                    nc.vector.tensor_copy(qtb[:, 0:512], ps1[:, :])
                    nc.vector.tensor_copy(qtb[:, 512:768], ps2[:, 0:256])
                else:
                    nc.scalar.copy(qtb[:, 0:512], ps1[:, :])
                    nc.scalar.copy(qtb[:, 512:768], ps2[:, 0:256])
                nc.tensor.matmul(
                    ps_gqt[0:L, 0:512],
                    gt_sb[:, st, :],
                    qtb[:, 0:512],
                    start=(st == 0),
                    stop=(st == ST - 1),
                )
                nc.tensor.matmul(
                    ps_gqt[0:L, 512:768],
                    gt_sb[:, st, :],
                    qtb[:, 512:768],
                    start=(st == 0),
                    stop=(st == ST - 1),
                )
            gqt_sb = small.tile([P, C], F32R, tag="gqt")
            nc.scalar.copy(gqt_sb[0:L, :], ps_gqt[0:L, 0:C])

            # logits^T = tp^T @ GQT per j-tile, fused exp -> E^T (f32r)
            et_sb = et_pool.tile([P, CT, C], F32R, tag="et")
            for jt in range(CT):
                psl = ps_big.tile([P, 1025], F32, tag="psb")
                lhsT = tp_sb[0:L, jt * P : (jt + 1) * P]
                nc.tensor.matmul(
                    psl[:, 0:512], lhsT, gqt_sb[0:L, 0:512], start=True, stop=True
                )
                nc.tensor.matmul(
                    psl[:, 512:768], lhsT, gqt_sb[0:L, 512:768], start=True, stop=True
                )
                nc.scalar.activation(et_sb[:, jt, :], psl[:, 0:C], EXP, scale=SCALE)

            # ZT = tp @ E^T  [L, C] accumulated over j-tiles
            ps_z1 = ps_small.tile([P, 512], F32, tag="ps")
            ps_z2 = ps_small.tile([P, 512], F32, tag="ps")
            for jt in range(CT):
                nc.tensor.matmul(
                    ps_z1[0:L, :],
                    tpT_sb[:, jt, 0:L],
                    et_sb[:, jt, 0:512],
                    start=(jt == 0),
                    stop=(jt == CT - 1),
                )
                nc.tensor.matmul(
                    ps_z2[0:L, 0:256],
                    tpT_sb[:, jt, 0:L],
                    et_sb[:, jt, 512:768],
                    start=(jt == 0),
                    stop=(jt == CT - 1),
                )
            zt_sb = small.tile([P, C], F32R, tag="zt")
            nc.scalar.copy(zt_sb[0:L, 0:512], ps_z1[0:L, :])
            nc.scalar.copy(zt_sb[0:L, 512:768], ps_z2[0:L, 0:256])

            # outA = ZT^T @ G (+ Z_i from ones column), epilogue, store
            for it in range(CT):
                psa = ps_big.tile([P, 1026], F32, tag="psb")
                lhsT = zt_sb[0:L, it * P : (it + 1) * P]
                nc.tensor.matmul(
                    psa[:, 0:512], lhsT, g_sb[0:L, 0:512], start=True, stop=True
                )
                nc.tensor.matmul(
                    psa[:, 512:1024], lhsT, g_sb[0:L, 512:1024], start=True, stop=True
                )
                for jt in range(CT):
                    nc.tensor.matmul(
                        psa[:, 1024:1026],
                        et_sb[:, jt, it * P : (it + 1) * P],
                        ones_sb[:],
                        start=(jt == 0),
                        stop=(jt == CT - 1),
                    )
                rz = zp.tile([P, 1], F32, tag="rz")
                nc.vector.reciprocal(rz[:], psa[:, 1024:1025])
                gz = zp.tile([P, 1], F32, tag="gz")
                nc.vector.tensor_scalar_mul(gz[:], rz[:], gamma_sb[:])
                o_sb = outp.tile([P, S], F32, tag="o")
                nc.vector.scalar_tensor_tensor(
                    o_sb[:], psa[:, 0:1024], gz[:], q_sb[:, it, :], op0=MULT, op1=ADD
                )
                nc.sync.dma_start(
                    out[b].rearrange("(ct p) s -> ct p s", p=P)[it], o_sb[:]
                )

    nc.compile()
    return nc


_NC = None


def _get_nc():
    global _NC
    if _NC is None:
        _NC = _build()
    return _NC


def _in_maps(img_feat, text_feat, W_txt, gamma):
    img = np.ascontiguousarray(img_feat.reshape(B, C, S), dtype=np.float32)
    txt = np.ascontiguousarray(text_feat, dtype=np.float32)
    wt = _round_tf32(np.ascontiguousarray(W_txt, dtype=np.float32))
    g = _round_tf32(_interp_matrix())
    gt = np.ascontiguousarray(g.T)
    gamma128 = np.full((P, 1), np.float32(gamma.reshape(-1)[0]), dtype=np.float32)
    maps = []
    for m in range(N_CORES):
        sl = slice(m * B_CORE, (m + 1) * B_CORE)
        maps.append(
            {
                "img": np.ascontiguousarray(img[sl]),
                "txt": np.ascontiguousarray(txt[sl]),
                "wt": wt,
                "g": g,
                "gt": gt,
                "gamma128": gamma128,
            }
        )
    return maps


def _run(in_maps, **kwargs):
    nc = _get_nc()
    return run_bass_kernel_spmd(nc, in_maps, core_ids=list(range(N_CORES)), **kwargs)


def kernel(img_feat, text_feat, W_txt, gamma):
    res = _run(_in_maps(img_feat, text_feat, W_txt, gamma))
    full = np.concatenate([res.results[m]["out"] for m in range(N_CORES)], axis=0)
    return full.reshape(B, C, HH, WW).astype(np.float32)



# revision 5
# speedup vs baseline: 1.4249x; 1.4249x over previous
"""Trainium2 Bass kernel: cross-modal channel attention (flipped-layout bf16).

Math (per batch b), with G the static [L, S] linear-interp matrix:
    qT   = img_feat[b]^T                          [S, C]   (host pre-transposed, bf16)
    tp   = text_feat[b] @ W_txt                   [L, C]
    GQT  = G @ qT                                 [L, C]
    logits^T = tp^T @ GQT * S^-0.5                [Cj, Ci]
    E^T  = exp(logits^T)                          [Cj, Ci]
    ZTa  = tpTa^T @ E^T                           [80, Ci]  (row 0 = Z via ones col)
    ZT'  = ZTa * (gamma / Z)                      [80, Ci]  (rank-1 replication matmul)
    out^T= qT + G_aug^T @ ZT'                     [S, C]    (residual via DVE/Pool adds)

Sharding: data-parallel over batch across 8 cores (4 batches/core);
consts replicated.  All matmul operands bf16 (PSUM accum fp32); I/O bf16.
Host does layout only: transpose/cast img->qT, text->txt^T, out^T->out.
"""

import sys

sys.path.insert(0, "/opt/trn_rl_repo")

from contextlib import ExitStack

import ml_dtypes
import numpy as np

import concourse.bacc as bacc
import concourse.mybir as mybir
import concourse.tile as tile
from concourse.bass_utils import run_bass_kernel_spmd
from concourse.masks import make_identity

B, C, HH, WW = 32, 768, 32, 32
S = HH * WW
L, D = 77, 512
N_CORES = 8
B_CORE = B // N_CORES
P = 128
CT, ST, DT = C // P, S // P, D // P
F32 = mybir.dt.float32
BF16 = mybir.dt.bfloat16
SCALE = float(S) ** -0.5
EXP = mybir.ActivationFunctionType.Exp
MULT = mybir.AluOpType.mult
ADD = mybir.AluOpType.add
NPBF = ml_dtypes.bfloat16

# Augmented-L layout: index 0 = ones/Z row, 1 = zero pad, 2..78 = l 0..76, 79 = pad.
LA = 80


def _interp_matrix():
    """G[l, s] such that (tp^T @ G)[c, s] == linear_interp(tp^T, S)[c, s]."""
    src = np.clip(
        (np.arange(S, dtype=np.float32) + np.float32(0.5)) * np.float32(L / S)
        - np.float32(0.5),
        np.float32(0.0),
        np.float32(L - 1),
    )
    i0 = np.floor(src).astype(np.int32)
    i1 = np.minimum(i0 + 1, L - 1)
    w = (src - i0.astype(np.float32)).astype(np.float32)
    g = np.zeros((L, S), dtype=np.float32)
    g[i0, np.arange(S)] += np.float32(1.0) - w
    g[i1, np.arange(S)] += w
    return g


def _build():
    nc = bacc.Bacc("TRN2", target_bir_lowering=False, debug=False)
    img = nc.dram_tensor("imgT", [B_CORE, S, C], BF16, kind="ExternalInput").ap()
    txt = nc.dram_tensor("txtT", [B_CORE, D, L], BF16, kind="ExternalInput").ap()
    wt = nc.dram_tensor("wt", [D, C], BF16, kind="ExternalInput").ap()
    g = nc.dram_tensor("ga", [LA, S], BF16, kind="ExternalInput").ap()
    gt = nc.dram_tensor("gt", [S, L], BF16, kind="ExternalInput").ap()
    gamma = nc.dram_tensor("gammarow", [1, LA], BF16, kind="ExternalInput").ap()
    out = nc.dram_tensor("outT", [B_CORE, S, C], BF16, kind="ExternalOutput").ap()

    with ExitStack() as ctx:
        ctx.enter_context(
            nc.allow_low_precision(reason="bf16 I/O fits the 2e-2 rel-err budget")
        )
        tc = ctx.enter_context(tile.TileContext(nc))
        consts = ctx.enter_context(tc.tile_pool(name="consts", bufs=1))
        q_pool = ctx.enter_context(tc.tile_pool(name="q", bufs=2))
        sb2 = ctx.enter_context(tc.tile_pool(name="sb2", bufs=2))
        et_pool = ctx.enter_context(tc.tile_pool(name="et", bufs=2))
        outp = ctx.enter_context(tc.tile_pool(name="outp", bufs=2))
        # PSUM: medium pool (tp/gqt/zt/zrep) 2x2 banks + big pool
        # (logits/trans/outA) 2x2 banks = 8 banks total.
        ps_med = ctx.enter_context(tc.tile_pool(name="ps_m", bufs=2, space="PSUM"))
        ps_big = ctx.enter_context(tc.tile_pool(name="ps_b", bufs=2, space="PSUM"))

        w_sb = consts.tile([P, DT, C], BF16)
        nc.sync.dma_start(w_sb[:], wt.rearrange("(k p) c -> p k c", p=P))
        g_sb = consts.tile([P, S], BF16)
        nc.sync.dma_start(g_sb[0:LA, :], g)
        gt_sb = consts.tile([P, ST, L], BF16)
        nc.sync.dma_start(gt_sb[:], gt.rearrange("(st p) l -> p st l", p=P))
        gamma_sb = consts.tile([P, LA], BF16)
        nc.sync.dma_start(gamma_sb[0:1, :], gamma)
        ident = consts.tile([P, P], F32)
        make_identity(nc, ident[:])
        ident_bf = consts.tile([P, P], BF16)
        nc.vector.tensor_copy(ident_bf[:], ident[:])

        for b in range(B_CORE):
            qT_sb = q_pool.tile([P, ST, C], BF16, tag="q")
            nc.sync.dma_start(qT_sb[:], img[b].rearrange("(st p) c -> p st c", p=P))
            txt_sb = sb2.tile([P, DT, L], BF16, tag="txt")
            nc.sync.dma_start(txt_sb[:], txt[b].rearrange("(k p) l -> p k l", p=P))

            # tp = text @ W_txt  [L, C]
            ps_tp = ps_med.tile([P, C], F32, tag="med")
            for k in range(DT):
                nc.tensor.matmul(
                    ps_tp[0:L, 0:512],
                    txt_sb[:, k, :],
                    w_sb[:, k, 0:512],
                    start=(k == 0),
                    stop=(k == DT - 1),
                )
            for k in range(DT):
                nc.tensor.matmul(
                    ps_tp[0:L, 512:768],
                    txt_sb[:, k, :],
                    w_sb[:, k, 512:768],
                    start=(k == 0),
                    stop=(k == DT - 1),
                )
            tp_sb = sb2.tile([P, C], BF16, tag="tp")
            nc.scalar.copy(tp_sb[0:L, :], ps_tp[0:L, :])

            # GQT = G @ qT  [L, C] accumulated over S-chunks
            ps_gqt = ps_med.tile([P, C], F32, tag="med")
            for st in range(ST):
                nc.tensor.matmul(
                    ps_gqt[0:L, 0:512],
                    gt_sb[:, st, :],
                    qT_sb[:, st, 0:512],
                    start=(st == 0),
                    stop=(st == ST - 1),
                )
            for st in range(ST):
                nc.tensor.matmul(
                    ps_gqt[0:L, 512:768],
                    gt_sb[:, st, :],
                    qT_sb[:, st, 512:768],
                    start=(st == 0),
                    stop=(st == ST - 1),
                )
            gqt_sb = sb2.tile([P, C], BF16, tag="gqt")
            nc.scalar.copy(gqt_sb[0:L, :], ps_gqt[0:L, :])

            # logits^T per j-tile, fused exp -> E^T (bf16)
            et_sb = et_pool.tile([P, CT, C], BF16, tag="et")
            for jt in range(CT):
                psl = ps_big.tile([P, C], F32, tag="big")
                lhsT = tp_sb[0:L, jt * P : (jt + 1) * P]
                nc.tensor.matmul(
                    psl[:, 0:512], lhsT, gqt_sb[0:L, 0:512], start=True, stop=True
                )
                nc.tensor.matmul(
                    psl[:, 512:768], lhsT, gqt_sb[0:L, 512:768], start=True, stop=True
                )
                nc.scalar.activation(et_sb[:, jt, :], psl[:, 0:C], EXP, scale=SCALE)

            # tp^T (augmented: col 0 = ones for Z, col 1 zero, 2..78 = tp^T, 79 zero)
            ps_tr = ps_big.tile([P, CT, LA], BF16, tag="big")
            for jt in range(CT):
                nc.tensor.transpose(
                    ps_tr[:, jt, 2 : 2 + 78],
                    tp_sb[0:L, jt * P : (jt + 1) * P],
                    ident_bf[0:L, 0:78],
                )
            tpa_sb = sb2.tile([P, CT, LA], BF16, tag="tpa")
            nc.vector.tensor_copy(tpa_sb[:, :, 2:80], ps_tr[:, :, 2:80])
            nc.gpsimd.memset(tpa_sb[:, :, 0:1], 1.0)
            nc.gpsimd.memset(tpa_sb[:, :, 1:2], 0.0)

            # ZTa = tpTa^T @ E^T  [LA, C]; row 0 = Z (softmax denominator)
            ps_zt = ps_med.tile([P, C], F32, tag="med")
            for jt in range(CT):
                nc.tensor.matmul(
                    ps_zt[0:LA, 0:512],
                    tpa_sb[:, jt, :],
                    et_sb[:, jt, 0:512],
                    start=(jt == 0),
                    stop=(jt == CT - 1),
                )
            for jt in range(CT):
                nc.tensor.matmul(
                    ps_zt[0:LA, 512:768],
                    tpa_sb[:, jt, :],
                    et_sb[:, jt, 512:768],
                    start=(jt == 0),
                    stop=(jt == CT - 1),
                )

            # gz = 1/Z (bf16 row); Zrep = gamma_row^T @ gz  [LA, C] = gamma/Z
            gz_sb = sb2.tile([P, C], BF16, tag="gz")
            nc.vector.reciprocal(gz_sb[0:1, :], ps_zt[0:1, :])
            ps_zrep = ps_med.tile([P, C], F32, tag="med")
            nc.tensor.matmul(
                ps_zrep[0:LA, 0:512],
                gamma_sb[0:1, :],
                gz_sb[0:1, 0:512],
                start=True,
                stop=True,
            )
            nc.tensor.matmul(
                ps_zrep[0:LA, 512:768],
                gamma_sb[0:1, :],
                gz_sb[0:1, 512:768],
                start=True,
                stop=True,
            )
            # ZT' = ZTa * (gamma/Z)  (ZTa staged through SBUF: DVE can't read 2 PSUMs)
            zt_sb = sb2.tile([P, C], BF16, tag="zt")
            nc.scalar.copy(zt_sb[0:LA, :], ps_zt[0:LA, :])
            ztp_sb = sb2.tile([P, C], BF16, tag="ztp")
            nc.vector.tensor_tensor(
                ztp_sb[0:LA, :], ps_zrep[0:LA, :], zt_sb[0:LA, :], op=MULT
            )

            # out^T per s-tile: psum = G_aug^T @ ZT', then += qT via DVE/Pool adds
            out_sb = outp.tile([P, ST, C], BF16, tag="o")
            for st in range(ST):
                pso = ps_big.tile([P, C], F32, tag="big")
                lhsT = g_sb[0:LA, st * P : (st + 1) * P]
                nc.tensor.matmul(
                    pso[:, 0:512], lhsT, ztp_sb[0:LA, 0:512], start=True, stop=True
                )
                nc.tensor.matmul(
                    pso[:, 512:768], lhsT, ztp_sb[0:LA, 512:768], start=True, stop=True
                )
                nc.vector.tensor_tensor(
                    out_sb[:, st, :], pso[:, 0:C], qT_sb[:, st, :], op=ADD
                )
            nc.sync.dma_start(
                out[b].rearrange("(st p) c -> p st c", p=P), out_sb[:]
            )

    nc.compile()
    return nc


_NC = None


def _get_nc():
    global _NC
    if _NC is None:
        _NC = _build()
    return _NC


def _in_maps(img_feat, text_feat, W_txt, gamma):
    imgT = (
        np.ascontiguousarray(img_feat, dtype=np.float32)
        .reshape(B, C, S)
        .transpose(0, 2, 1)
        .astype(NPBF)
    )
    txtT = (
        np.ascontiguousarray(text_feat, dtype=np.float32)
        .transpose(0, 2, 1)
        .astype(NPBF)
    )
    wt = np.ascontiguousarray(W_txt, dtype=np.float32).astype(NPBF)
    g = _interp_matrix()
    ga = np.zeros((LA, S), dtype=np.float32)
    ga[2 : 2 + L] = g
    ga = ga.astype(NPBF)
    gt = np.ascontiguousarray(g.T).astype(NPBF)
    gammarow = np.full((1, LA), np.float32(gamma.reshape(-1)[0]), dtype=np.float32)
    gammarow = gammarow.astype(NPBF)
    maps = []
    for m in range(N_CORES):
        sl = slice(m * B_CORE, (m + 1) * B_CORE)
        maps.append(
            {
                "imgT": np.ascontiguousarray(imgT[sl]),
                "txtT": np.ascontiguousarray(txtT[sl]),
                "wt": wt,
                "ga": ga,
                "gt": gt,
                "gammarow": gammarow,
            }
        )
    return maps


def _run(in_maps, **kwargs):
    nc = _get_nc()
    return run_bass_kernel_spmd(nc, in_maps, core_ids=list(range(N_CORES)), **kwargs)


def kernel(img_feat, text_feat, W_txt, gamma):
    res = _run(_in_maps(img_feat, text_feat, W_txt, gamma))
    full = np.concatenate(
        [np.asarray(res.results[m]["outT"]) for m in range(N_CORES)], axis=0
    )
    full = full.astype(np.float32).transpose(0, 2, 1)
    return np.ascontiguousarray(full).reshape(B, C, HH, WW)


# revision 7
# speedup vs baseline: 1.5981x; 1.1215x over previous
"""Trainium2 Bass kernel: cross-modal channel attention (flipped-layout bf16,
software-pipelined across batches).

Math (per batch b), with G the static [L, S] linear-interp matrix:
    qT   = img_feat[b]^T                          [S, C]   (host pre-transposed, bf16)
    tp   = text_feat[b] @ W_txt                   [L, C]
    GQT  = G @ qT                                 [L, C]
    logits^T = tp^T @ GQT * S^-0.5                [Cj, Ci]
    E^T  = exp(logits^T)                          [Cj, Ci]
    ZTa  = tpTa^T @ E^T                           [80, Ci]  (row 0 = Z via ones col)
    ZT'  = (gamma * ZTa) * bcast(1/Z)             [80, Ci]
    out^T= qT + G_aug^T @ ZT'                     [S, C]    (residual via DVE adds)

Three-stage software pipeline over the 4 per-core batches so the PE never
stalls on the Act/DVE conversions between matmul phases:
    iter i: front(i)=tp+GQT mms | mid(i-1)=trans+logits+exp+ZT | back(i-2)=outA+resid
Sharding: data-parallel over batch across 8 cores; consts replicated.
Host does layout only: transpose/cast img->qT, text->txt^T, out^T->out.
"""

import sys

sys.path.insert(0, "/opt/trn_rl_repo")

from contextlib import ExitStack

import ml_dtypes
import numpy as np

import concourse.bacc as bacc
import concourse.mybir as mybir
import concourse.tile as tile
from concourse.bass_utils import run_bass_kernel_spmd
from concourse.masks import make_identity

B, C, HH, WW = 32, 768, 32, 32
S = HH * WW
L, D = 77, 512
N_CORES = 8
B_CORE = B // N_CORES
P = 128
CT, ST, DT = C // P, S // P, D // P
F32 = mybir.dt.float32
BF16 = mybir.dt.bfloat16
SCALE = float(S) ** -0.5
EXP = mybir.ActivationFunctionType.Exp
MULT = mybir.AluOpType.mult
ADD = mybir.AluOpType.add
NPBF = ml_dtypes.bfloat16

# Augmented-L layout: col/row 0 = ones/Z, 1 = zero, 2..78 = l 0..76, 79 = zero.
LA = 80


def _interp_matrix():
    """G[l, s] such that (tp^T @ G)[c, s] == linear_interp(tp^T, S)[c, s]."""
    src = np.clip(
        (np.arange(S, dtype=np.float32) + np.float32(0.5)) * np.float32(L / S)
        - np.float32(0.5),
        np.float32(0.0),
        np.float32(L - 1),
    )
    i0 = np.floor(src).astype(np.int32)
    i1 = np.minimum(i0 + 1, L - 1)
    w = (src - i0.astype(np.float32)).astype(np.float32)
    g = np.zeros((L, S), dtype=np.float32)
    g[i0, np.arange(S)] += np.float32(1.0) - w
    g[i1, np.arange(S)] += w
    return g


def _build():
    nc = bacc.Bacc("TRN2", target_bir_lowering=False, debug=False)
    img = nc.dram_tensor("imgT", [B_CORE, S, C], BF16, kind="ExternalInput").ap()
    txt = nc.dram_tensor("txtT", [B_CORE, D, L], BF16, kind="ExternalInput").ap()
    wt = nc.dram_tensor("wt", [D, C], BF16, kind="ExternalInput").ap()
    g = nc.dram_tensor("ga", [LA, S], BF16, kind="ExternalInput").ap()
    gt = nc.dram_tensor("gt", [S, L], BF16, kind="ExternalInput").ap()
    gamma = nc.dram_tensor("gammacol", [P, 1], F32, kind="ExternalInput").ap()
    out = nc.dram_tensor("outT", [B_CORE, S, C], BF16, kind="ExternalOutput").ap()

    with ExitStack() as ctx:
        ctx.enter_context(
            nc.allow_low_precision(reason="bf16 I/O fits the 2e-2 rel-err budget")
        )
        tc = ctx.enter_context(tile.TileContext(nc))
        consts = ctx.enter_context(tc.tile_pool(name="consts", bufs=1))
        q_pool = ctx.enter_context(tc.tile_pool(name="q", bufs=5))
        txt_pool = ctx.enter_context(tc.tile_pool(name="txtp", bufs=3))
        sb2 = ctx.enter_context(tc.tile_pool(name="sb2", bufs=2))
        et_pool = ctx.enter_context(tc.tile_pool(name="et", bufs=2))
        outp = ctx.enter_context(tc.tile_pool(name="outp", bufs=2))
        # PSUM: med tag (tp/gqt/trans/zt) 2x2 banks + big tag (logits/outA)
        # 2x2 banks = 8 banks total.
        ps_med = ctx.enter_context(tc.tile_pool(name="ps_m", bufs=2, space="PSUM"))
        ps_big = ctx.enter_context(tc.tile_pool(name="ps_b", bufs=2, space="PSUM"))

        w_sb = consts.tile([P, DT, C], BF16)
        nc.sync.dma_start(w_sb[:], wt.rearrange("(k p) c -> p k c", p=P))
        g_sb = consts.tile([P, S], BF16)
        nc.sync.dma_start(g_sb[0:LA, :], g)
        gt_sb = consts.tile([P, ST, L], BF16)
        nc.sync.dma_start(gt_sb[:], gt.rearrange("(st p) l -> p st l", p=P))
        gamma_sb = consts.tile([P, 1], F32)
        nc.sync.dma_start(gamma_sb[:], gamma)
        ident = consts.tile([P, P], F32)
        make_identity(nc, ident[:])
        ident_bf = consts.tile([P, P], BF16)
        nc.vector.tensor_copy(ident_bf[:], ident[:])

        qT = [None] * B_CORE
        txts = [None] * B_CORE
        tp_sb = [None] * B_CORE
        gqt_sb = [None] * B_CORE
        et_sb = [None] * B_CORE
        tpa_sb = [None] * B_CORE
        zt_sb = [None] * B_CORE
        gz_sb = [None] * B_CORE
        gzb_sb = [None] * B_CORE
        ztp_sb = [None] * B_CORE

        def dmas(b):
            qT[b] = q_pool.tile([P, ST, C], BF16, tag="q", name=f"qT{b}")
            nc.sync.dma_start(qT[b][:], img[b].rearrange("(st p) c -> p st c", p=P))
            txts[b] = txt_pool.tile([P, DT, L], BF16, tag="txt", name=f"txts{b}")
            nc.sync.dma_start(txts[b][:], txt[b].rearrange("(k p) l -> p k l", p=P))

        def front(b):
            # tp = text @ W_txt [L, C]
            ps_tp = ps_med.tile([P, C], F32, tag="med")
            for half, (c0, c1) in enumerate(((0, 512), (512, 768))):
                for k in range(DT):
                    nc.tensor.matmul(
                        ps_tp[0:L, c0:c1],
                        txts[b][:, k, :],
                        w_sb[:, k, c0:c1],
                        start=(k == 0),
                        stop=(k == DT - 1),
                    )
            tp_sb[b] = sb2.tile([P, C], BF16, tag="tp", name=f"tp{b}")
            nc.scalar.copy(tp_sb[b][0:L, :], ps_tp[0:L, :])
            # GQT = G @ qT [L, C]
            ps_gqt = ps_med.tile([P, C], F32, tag="med")
            for c0, c1 in ((0, 512), (512, 768)):
                for st in range(ST):
                    nc.tensor.matmul(
                        ps_gqt[0:L, c0:c1],
                        gt_sb[:, st, :],
                        qT[b][:, st, c0:c1],
                        start=(st == 0),
                        stop=(st == ST - 1),
                    )
            gqt_sb[b] = sb2.tile([P, C], BF16, tag="gqt", name=f"gqt{b}")
            nc.scalar.copy(gqt_sb[b][0:L, :], ps_gqt[0:L, :])

        def mid(b):
            # tp^T (augmented): col 0 ones, col 1 zero, cols 2:80 = tp^T + zero pad
            ps_tr = ps_med.tile([P, CT, LA], BF16, tag="med")
            for jt in range(CT):
                nc.tensor.transpose(
                    ps_tr[:, jt, 2:80],
                    tp_sb[b][0:L, jt * P : (jt + 1) * P],
                    ident_bf[0:L, 0:78],
                )
            tpa_sb[b] = sb2.tile([P, CT, LA], BF16, tag="tpa", name=f"tpa{b}")
            nc.gpsimd.memset(tpa_sb[b][:, :, 0:1], 1.0)
            nc.gpsimd.memset(tpa_sb[b][:, :, 1:2], 0.0)
            nc.scalar.copy(tpa_sb[b][:, :, 2:80], ps_tr[:, :, 2:80])
            # logits^T per j-tile, fused exp -> E^T (bf16)
            et_sb[b] = et_pool.tile([P, CT, C], BF16, tag="et", name=f"et{b}")
            for jt in range(CT):
                psl = ps_big.tile([P, C], F32, tag="big")
                lhsT = tp_sb[b][0:L, jt * P : (jt + 1) * P]
                for c0, c1 in ((0, 512), (512, 768)):
                    nc.tensor.matmul(
                        psl[:, c0:c1],
                        lhsT,
                        gqt_sb[b][0:L, c0:c1],
                        start=True,
                        stop=True,
                    )
                nc.scalar.activation(et_sb[b][:, jt, :], psl[:, 0:C], EXP, scale=SCALE)
            # ZTa = tpTa^T @ E^T [LA, C]; row 0 = Z
            ps_zt = ps_med.tile([P, C], F32, tag="med")
            for c0, c1 in ((0, 512), (512, 768)):
                for jt in range(CT):
                    nc.tensor.matmul(
                        ps_zt[0:LA, c0:c1],
                        tpa_sb[b][:, jt, :],
                        et_sb[b][:, jt, c0:c1],
                        start=(jt == 0),
                        stop=(jt == CT - 1),
                    )
            # 1/Z row; gamma*ZTa (gamma folded into the PSUM->SBUF conv scale)
            gz_sb[b] = sb2.tile([P, C], BF16, tag="gz", name=f"gz{b}")
            nc.vector.reciprocal(gz_sb[b][0:1, :], ps_zt[0:1, :])
            zt_sb[b] = sb2.tile([P, C], BF16, tag="zt", name=f"zt{b}")
            nc.scalar.activation(
                zt_sb[b][0:LA, :],
                ps_zt[0:LA, :],
                mybir.ActivationFunctionType.Copy,
                scale=gamma_sb[0:LA, :],
            )
            gzb_sb[b] = sb2.tile([P, C], BF16, tag="gzb", name=f"gzb{b}")
            nc.gpsimd.partition_broadcast(gzb_sb[b][0:LA, :], gz_sb[b][0:1, :])

        def backpre(b):
            # ZT' = (gamma*ZTa) * bcast(1/Z)   (all-SBUF bf16 -> DVE 2x eligible)
            ztp_sb[b] = sb2.tile([P, C], BF16, tag="ztp", name=f"ztp{b}")
            nc.vector.tensor_tensor(
                ztp_sb[b][0:LA, :], zt_sb[b][0:LA, :], gzb_sb[b][0:LA, :], op=MULT
            )

        def back(b):
            out_sb = outp.tile([P, ST, C], BF16, tag="o")
            for st in range(ST):
                pso = ps_big.tile([P, C], F32, tag="big")
                lhsT = g_sb[0:LA, st * P : (st + 1) * P]
                for c0, c1 in ((0, 512), (512, 768)):
                    nc.tensor.matmul(
                        pso[:, c0:c1],
                        lhsT,
                        ztp_sb[b][0:LA, c0:c1],
                        start=True,
                        stop=True,
                    )
                nc.vector.tensor_tensor(
                    out_sb[:, st, :], pso[:, 0:C], qT[b][:, st, :], op=ADD
                )
            nc.sync.dma_start(out[b].rearrange("(st p) c -> p st c", p=P), out_sb[:])

        dmas(0)
        if B_CORE > 1:
            dmas(1)
        for i in range(B_CORE + 2):
            if i + 2 < B_CORE:
                dmas(i + 2)
            if i < B_CORE:
                front(i)
            if i >= 2:
                backpre(i - 2)
            if 1 <= i <= B_CORE:
                mid(i - 1)
            if i >= 2:
                back(i - 2)

    nc.compile()
    return nc


_NC = None


def _get_nc():
    global _NC
    if _NC is None:
        _NC = _build()
    return _NC


def _in_maps(img_feat, text_feat, W_txt, gamma):
    imgT = (
        np.ascontiguousarray(img_feat, dtype=np.float32)
        .reshape(B, C, S)
        .transpose(0, 2, 1)
        .astype(NPBF)
    )
    txtT = (
        np.ascontiguousarray(text_feat, dtype=np.float32)
        .transpose(0, 2, 1)
        .astype(NPBF)
    )
    wt = np.ascontiguousarray(W_txt, dtype=np.float32).astype(NPBF)
    g = _interp_matrix()
    ga = np.zeros((LA, S), dtype=np.float32)
    ga[2 : 2 + L] = g
    ga = ga.astype(NPBF)
    gt = np.ascontiguousarray(g.T).astype(NPBF)
    gammacol = np.full((P, 1), np.float32(gamma.reshape(-1)[0]), dtype=np.float32)
    maps = []
    for m in range(N_CORES):
        sl = slice(m * B_CORE, (m + 1) * B_CORE)
        maps.append(
            {
                "imgT": np.ascontiguousarray(imgT[sl]),
                "txtT": np.ascontiguousarray(txtT[sl]),
                "wt": wt,
                "ga": ga,
                "gt": gt,
                "gammacol": gammacol,
            }
        )
    return maps


def _run(in_maps, **kwargs):
    nc = _get_nc()
    return run_bass_kernel_spmd(nc, in_maps, core_ids=list(range(N_CORES)), **kwargs)


def kernel(img_feat, text_feat, W_txt, gamma):
    res = _run(_in_maps(img_feat, text_feat, W_txt, gamma))
    full = np.concatenate(
        [np.asarray(res.results[m]["outT"]) for m in range(N_CORES)], axis=0
    )
    full = full.astype(np.float32).transpose(0, 2, 1)
    return np.ascontiguousarray(full).reshape(B, C, HH, WW)


# revision 11
# speedup vs baseline: 1.7668x; 1.1056x over previous
"""Trainium2 Bass kernel: cross-modal channel attention (flipped-layout bf16,
software-pipelined across batches).

Math (per batch b), with G the static [L, S] linear-interp matrix:
    qT   = img_feat[b]^T                          [S, C]   (host pre-transposed, bf16)
    tp   = text_feat[b] @ W_txt                   [L, C]
    GQT  = G @ qT                                 [L, C]
    logits^T = tp^T @ GQT * S^-0.5                [Cj, Ci]
    E^T  = exp(logits^T)                          [Cj, Ci]
    ZTa  = tpTa^T @ E^T                           [80, Ci]  (row 0 = Z via ones col)
    ZT'  = (gamma * ZTa) * bcast(1/Z)             [80, Ci]
    out^T= qT + G_aug^T @ ZT'                     [S, C]    (residual via DVE adds)

Three-stage software pipeline over the 4 per-core batches so the PE never
stalls on the Act/DVE conversions between matmul phases:
    iter i: front(i)=tp+GQT mms | mid(i-1)=trans+logits+exp+ZT | back(i-2)=outA+resid
Sharding: data-parallel over batch across 8 cores; consts replicated.
Host does layout only: transpose/cast img->qT, text->txt^T, out^T->out.
"""

import sys

sys.path.insert(0, "/opt/trn_rl_repo")

from contextlib import ExitStack

import ml_dtypes
import numpy as np

import concourse.bacc as bacc
import concourse.mybir as mybir
import concourse.tile as tile
from concourse.bass_utils import run_bass_kernel_spmd
from concourse.masks import make_identity

B, C, HH, WW = 32, 768, 32, 32
S = HH * WW
L, D = 77, 512
N_CORES = 8
B_CORE = B // N_CORES
P = 128
CT, ST, DT = C // P, S // P, D // P
F32 = mybir.dt.float32
BF16 = mybir.dt.bfloat16
SCALE = float(S) ** -0.5
EXP = mybir.ActivationFunctionType.Exp
MULT = mybir.AluOpType.mult
ADD = mybir.AluOpType.add
NPBF = ml_dtypes.bfloat16

# Augmented-L layout: col/row 0 = ones/Z, 1 = zero, 2..78 = l 0..76, 79 = zero.
LA = 80


def _interp_matrix():
    """G[l, s] such that (tp^T @ G)[c, s] == linear_interp(tp^T, S)[c, s]."""
    src = np.clip(
        (np.arange(S, dtype=np.float32) + np.float32(0.5)) * np.float32(L / S)
        - np.float32(0.5),
        np.float32(0.0),
        np.float32(L - 1),
    )
    i0 = np.floor(src).astype(np.int32)
    i1 = np.minimum(i0 + 1, L - 1)
    w = (src - i0.astype(np.float32)).astype(np.float32)
    g = np.zeros((L, S), dtype=np.float32)
    g[i0, np.arange(S)] += np.float32(1.0) - w
    g[i1, np.arange(S)] += w
    return g


def _build():
    nc = bacc.Bacc("TRN2", target_bir_lowering=False, debug=False)
    img = nc.dram_tensor("imgT", [B_CORE, S, C], BF16, kind="ExternalInput").ap()
    txt = nc.dram_tensor("txtT", [B_CORE, D, L], BF16, kind="ExternalInput").ap()
    wt = nc.dram_tensor("wt", [D, C], BF16, kind="ExternalInput").ap()
    g = nc.dram_tensor("ga", [LA, S], BF16, kind="ExternalInput").ap()
    gt = nc.dram_tensor("gt", [S, L], BF16, kind="ExternalInput").ap()
    gamma = nc.dram_tensor("gammacol", [P, 1], F32, kind="ExternalInput").ap()
    out = nc.dram_tensor("outT", [B_CORE, S, C], BF16, kind="ExternalOutput").ap()

    with ExitStack() as ctx:
        ctx.enter_context(
            nc.allow_low_precision(reason="bf16 I/O fits the 2e-2 rel-err budget")
        )
        tc = ctx.enter_context(tile.TileContext(nc))
        consts = ctx.enter_context(tc.tile_pool(name="consts", bufs=1))
        q_pool = ctx.enter_context(tc.tile_pool(name="q", bufs=5))
        txt_pool = ctx.enter_context(tc.tile_pool(name="txtp", bufs=3))
        sb2 = ctx.enter_context(tc.tile_pool(name="sb2", bufs=2))
        et_pool = ctx.enter_context(tc.tile_pool(name="et", bufs=2))
        outp = ctx.enter_context(tc.tile_pool(name="outp", bufs=2))
        # PSUM: med tag (tp/gqt/trans/zt) 2x2 banks + big tag (logits/outA)
        # 2x2 banks = 8 banks total.
        ps_med = ctx.enter_context(tc.tile_pool(name="ps_m", bufs=2, space="PSUM"))
        ps_big = ctx.enter_context(tc.tile_pool(name="ps_b", bufs=2, space="PSUM"))

        qT = [None] * B_CORE
        txts = [None] * B_CORE
        tp_sb = [None] * B_CORE
        gqt_sb = [None] * B_CORE
        et_sb = [None] * B_CORE
        tpa_sb = [None] * B_CORE
        zt_sb = [None] * B_CORE
        gz_sb = [None] * B_CORE
        gzb_sb = [None] * B_CORE
        ztp_sb = [None] * B_CORE

        def dma_txt(b):
            txts[b] = txt_pool.tile([P, DT, L], BF16, tag="txt", name=f"txts{b}")
            nc.sync.dma_start(txts[b][:], txt[b].rearrange("(k p) l -> p k l", p=P))

        def dma_q(b):
            qT[b] = q_pool.tile([P, ST, C], BF16, tag="q", name=f"qT{b}")
            nc.sync.dma_start(qT[b][:], img[b].rearrange("(st p) c -> p st c", p=P))

        def dmas(b):
            dma_q(b)
            dma_txt(b)

        # DMA issue order = transfer order (single DMA lane): feed the first
        # compute phases first, bulky consts later.
        dma_txt(0)
        w_sb = consts.tile([P, DT, C], BF16)
        nc.sync.dma_start(w_sb[:], wt.rearrange("(k p) c -> p k c", p=P))
        dma_q(0)
        gt_sb = consts.tile([P, ST, L], BF16)
        nc.sync.dma_start(gt_sb[:], gt.rearrange("(st p) l -> p st l", p=P))
        dma_txt(1)
        dma_q(1)
        g_sb = consts.tile([P, S], BF16)
        nc.sync.dma_start(g_sb[0:LA, :], g)
        gamma_sb = consts.tile([P, 1], F32)
        nc.sync.dma_start(gamma_sb[:], gamma)
        ident = consts.tile([P, P], F32)
        make_identity(nc, ident[:])
        ident_bf = consts.tile([P, P], BF16)
        nc.vector.tensor_copy(ident_bf[:], ident[:])

        def front(b):
            # tp = text @ W_txt [L, C]
            ps_tp = ps_med.tile([P, C], F32, tag="med")
            for half, (c0, c1) in enumerate(((0, 512), (512, 768))):
                for k in range(DT):
                    nc.tensor.matmul(
                        ps_tp[0:L, c0:c1],
                        txts[b][:, k, :],
                        w_sb[:, k, c0:c1],
                        start=(k == 0),
                        stop=(k == DT - 1),
                    )
            tp_sb[b] = sb2.tile([P, C], BF16, tag="tp", name=f"tp{b}")
            nc.vector.tensor_copy(tp_sb[b][0:L, :], ps_tp[0:L, :])
            # GQT = G @ qT [L, C]
            ps_gqt = ps_med.tile([P, C], F32, tag="med")
            for c0, c1 in ((0, 512), (512, 768)):
                for st in range(ST):
                    nc.tensor.matmul(
                        ps_gqt[0:L, c0:c1],
                        gt_sb[:, st, :],
                        qT[b][:, st, c0:c1],
                        start=(st == 0),
                        stop=(st == ST - 1),
                    )
            gqt_sb[b] = sb2.tile([P, C], BF16, tag="gqt", name=f"gqt{b}")
            nc.vector.tensor_copy(gqt_sb[b][0:L, :], ps_gqt[0:L, :])

        def mid(b):
            # tp^T (augmented): col 0 ones, col 1 zero, cols 2:80 = tp^T + zero pad
            ps_tr = ps_med.tile([P, CT, LA], BF16, tag="med")
            for jt in range(CT):
                nc.tensor.transpose(
                    ps_tr[:, jt, 2:80],
                    tp_sb[b][0:L, jt * P : (jt + 1) * P],
                    ident_bf[0:L, 0:78],
                )
            tpa_sb[b] = sb2.tile([P, CT, LA], BF16, tag="tpa", name=f"tpa{b}")
            nc.gpsimd.memset(tpa_sb[b][:, :, 0:1], 1.0)
            nc.gpsimd.memset(tpa_sb[b][:, :, 1:2], 0.0)
            nc.vector.tensor_copy(tpa_sb[b][:, :, 2:80], ps_tr[:, :, 2:80])
            # logits^T per j-tile, fused exp -> E^T (bf16)
            et_sb[b] = et_pool.tile([P, CT, C], BF16, tag="et", name=f"et{b}")
            for jt in range(CT):
                psl = ps_big.tile([P, C], F32, tag="big")
                lhsT = tp_sb[b][0:L, jt * P : (jt + 1) * P]
                for c0, c1 in ((0, 512), (512, 768)):
                    nc.tensor.matmul(
                        psl[:, c0:c1],
                        lhsT,
                        gqt_sb[b][0:L, c0:c1],
                        start=True,
                        stop=True,
                    )
                nc.scalar.activation(et_sb[b][:, jt, :], psl[:, 0:C], EXP, scale=SCALE)
            # ZTa = tpTa^T @ E^T [LA, C]; row 0 = Z
            ps_zt = ps_med.tile([P, C], F32, tag="med")
            for c0, c1 in ((0, 512), (512, 768)):
                for jt in range(CT):
                    nc.tensor.matmul(
                        ps_zt[0:LA, c0:c1],
                        tpa_sb[b][:, jt, :],
                        et_sb[b][:, jt, c0:c1],
                        start=(jt == 0),
                        stop=(jt == CT - 1),
                    )
            # 1/Z row; gamma*ZTa (gamma folded into the PSUM->SBUF conv scale)
            gz_sb[b] = sb2.tile([P, C], BF16, tag="gz", name=f"gz{b}")
            nc.vector.reciprocal(gz_sb[b][0:1, :], ps_zt[0:1, :])
            zt_sb[b] = sb2.tile([P, C], BF16, tag="zt", name=f"zt{b}")
            nc.scalar.activation(
                zt_sb[b][0:LA, :],
                ps_zt[0:LA, :],
                mybir.ActivationFunctionType.Copy,
                scale=gamma_sb[0:LA, :],
            )
            gzb_sb[b] = sb2.tile([P, C], BF16, tag="gzb", name=f"gzb{b}")
            nc.gpsimd.partition_broadcast(gzb_sb[b][0:LA, :], gz_sb[b][0:1, :])

        def backpre(b):
            # ZT' = (gamma*ZTa) * bcast(1/Z)   (all-SBUF bf16 -> DVE 2x eligible)
            ztp_sb[b] = sb2.tile([P, C], BF16, tag="ztp", name=f"ztp{b}")
            nc.vector.tensor_tensor(
                ztp_sb[b][0:LA, :], zt_sb[b][0:LA, :], gzb_sb[b][0:LA, :], op=MULT
            )

        ACT_ST = (1, 4, 6)  # these s-tiles add the residual on PE, convert on Act

        def back(b):
            out_sb = outp.tile([P, ST, C], BF16, tag="o")
            for st in range(ST):
                pso = ps_big.tile([P, C], F32, tag="big")
                lhsT = g_sb[0:LA, st * P : (st + 1) * P]
                on_act = st in ACT_ST
                for c0, c1 in ((0, 512), (512, 768)):
                    nc.tensor.matmul(
                        pso[:, c0:c1],
                        lhsT,
                        ztp_sb[b][0:LA, c0:c1],
                        start=True,
                        stop=not on_act,
                    )
                    if on_act:
                        nc.tensor.matmul(
                            pso[:, c0:c1],
                            ident_bf[:],
                            qT[b][:, st, c0:c1],
                            start=False,
                            stop=True,
                        )
                if on_act:
                    nc.scalar.copy(out_sb[:, st, :], pso[:, 0:C])
                else:
                    nc.vector.tensor_tensor(
                        out_sb[:, st, :], pso[:, 0:C], qT[b][:, st, :], op=ADD
                    )
            nc.sync.dma_start(out[b].rearrange("(st p) c -> p st c", p=P), out_sb[:])

        for i in range(B_CORE + 2):
            if i + 2 < B_CORE:
                dmas(i + 2)
            if i < B_CORE:
                front(i)
            if i >= 2:
                backpre(i - 2)
            if 1 <= i <= B_CORE:
                mid(i - 1)
            if i >= 2:
                back(i - 2)

    nc.compile()
    return nc


_NC = None


def _get_nc():
    global _NC
    if _NC is None:
        _NC = _build()
    return _NC


def _in_maps(img_feat, text_feat, W_txt, gamma):
    imgT = (
        np.ascontiguousarray(img_feat, dtype=np.float32)
        .reshape(B, C, S)
        .transpose(0, 2, 1)
        .astype(NPBF)
    )
    txtT = (
        np.ascontiguousarray(text_feat, dtype=np.float32)
        .transpose(0, 2, 1)
        .astype(NPBF)
    )
    wt = np.ascontiguousarray(W_txt, dtype=np.float32).astype(NPBF)
    g = _interp_matrix()
    ga = np.zeros((LA, S), dtype=np.float32)
    ga[2 : 2 + L] = g
    ga = ga.astype(NPBF)
    gt = np.ascontiguousarray(g.T).astype(NPBF)
    gammacol = np.full((P, 1), np.float32(gamma.reshape(-1)[0]), dtype=np.float32)
    maps = []
    for m in range(N_CORES):
        sl = slice(m * B_CORE, (m + 1) * B_CORE)
        maps.append(
            {
                "imgT": np.ascontiguousarray(imgT[sl]),
                "txtT": np.ascontiguousarray(txtT[sl]),
                "wt": wt,
                "ga": ga,
                "gt": gt,
                "gammacol": gammacol,
            }
        )
    return maps


def _run(in_maps, **kwargs):
    nc = _get_nc()
    return run_bass_kernel_spmd(nc, in_maps, core_ids=list(range(N_CORES)), **kwargs)


def kernel(img_feat, text_feat, W_txt, gamma):
    res = _run(_in_maps(img_feat, text_feat, W_txt, gamma))
    full = np.concatenate(
        [np.asarray(res.results[m]["outT"]) for m in range(N_CORES)], axis=0
    )
    full = full.astype(np.float32).transpose(0, 2, 1)
    return np.ascontiguousarray(full).reshape(B, C, HH, WW)
